# revision 27
# baseline (speedup 1.0000x reference)
import numpy as np
import ml_dtypes

import concourse.bacc as bacc
import concourse.mybir as mybir
import concourse.tile as tile
from concourse.bass_utils import run_bass_kernel_spmd

# Problem constants (hardcoded per harness contract)
B, H, W, C = 32, 32, 32, 128
NUM, D0, D1 = 10, 60, 16
JK = D0 * D1            # 960
OO = NUM * JK           # 9600
P = H * W               # 1024 contraction dim of the dense kernel
N_CORES = 8
B_LOC = B // N_CORES    # 4 batches per core
BLK = 512               # dense-kernel column block (1 PSUM bank of fp32)
NBLK = (OO + BLK - 1) // BLK  # 19, last block is 384 wide
BN = B_LOC * NUM        # 40
EPS = 1e-12
MAGIC = 0x5F3759DF      # quake rsqrt seed

# capsule i of a batch is fully drained once dense block TRIG[blk] lands
TRIG = {}
for i in range(NUM):
    _blk = ((i + 1) * JK + BLK - 1) // BLK - 1
    TRIG.setdefault(_blk, []).append(i)

f32 = mybir.dt.float32
i32 = mybir.dt.int32
f32r = mybir.dt.float32r
bf16 = mybir.dt.bfloat16
AF = mybir.ActivationFunctionType
ALU = mybir.AluOpType


def build_nc():
    nc = bacc.Bacc("TRN2", debug=False)
    u_d = nc.dram_tensor("u", (B_LOC, H, W, C), bf16, kind="ExternalInput").ap()
    wc_d = nc.dram_tensor("wc", (2, 2, C, C), bf16, kind="ExternalInput").ap()
    km_d = nc.dram_tensor("km", (P, OO), bf16, kind="ExternalInput").ap()
    eye_d = nc.dram_tensor("eye", (C, C), bf16, kind="ExternalInput").ap()
    out_d = nc.dram_tensor("out", (B_LOC, NUM, JK), f32, kind="ExternalOutput").ap()

    with tile.TileContext(nc) as tc:
        with tc.tile_pool(name="persist", bufs=1) as pers:
            u_hat = pers.tile([128, B_LOC, OO], bf16)       # [c, b, o]
            xin = pers.tile([128, B_LOC, 8, 128], bf16)     # raw u [sp, b, t, c]
            uT = pers.tile([128, B_LOC, 8, 128], bf16)      # lhsT chunks [p, b, chunk, c]
            wct = pers.tile([128, 4, C], bf16)              # conv taps [ci, tap, co]
            eye = pers.tile([128, C], bf16)
            ones_bf = pers.tile([128, 128], bf16)
            crep0 = pers.tile([128, 128], bf16)             # uniform c = 0.1
            ones32 = pers.tile([128, 128], f32r)
            c_all = pers.tile([128, B_LOC, NUM], f32)
            c_bf = pers.tile([128, B_LOC, NUM], bf16)
            z_all = pers.tile([128, BN], f32)
            zc = pers.tile([128, BN], f32r)
            ssm = pers.tile([128, BN], f32)
            magic = pers.tile([128, BN], i32)
            yr = pers.tile([128, BN], f32)
            ha = pers.tile([128, BN], f32)
            tmp = pers.tile([128, BN], f32)
            blog = pers.tile([128, BN], f32)
            eexp = pers.tile([128, B_LOC, NUM], f32)
            nmax = pers.tile([128, B_LOC], f32)
            sume = pers.tile([128, B_LOC], f32)
            rsum = pers.tile([128, B_LOC], f32)
            ofin = pers.tile([128, JK], f32)                # rows (b*NUM+i) hold final o
            # cdiag[:, r, :] = c2 column (b,i)=r at position r, zeros elsewhere;
            # used as t2 lhsT so capsule r lands on PSUM row r (base partition 0)
            cdiag = pers.tile([128, BN, BN], bf16)

            xpad = pers.tile([128, 33 * 33], bf16)
            zcol = pers.tile([128, 33], bf16)

            # input DMAs first, spread across queues so descriptor generation
            # overlaps; batch 0 first so conv can start as early as possible
            nc.sync.dma_start(eye[:], eye_d)
            uv = u_d.rearrange("b h w c -> b (h w) c").rearrange(
                "b (t sp) c -> b sp t c", sp=128)
            nc.gpsimd.dma_start(xin[:, 0], uv[0])
            nc.scalar.dma_start(wct[:], wc_d.rearrange("dh dw ci co -> ci (dh dw) co"))
            for _b in range(1, B_LOC):
                nc.gpsimd.dma_start(xin[:, _b], uv[_b])

            nc.vector.memset(ones_bf[:], 1.0)
            nc.vector.tensor_copy(ones32[:], ones_bf[:])
            nc.vector.memset(crep0[:], 0.1)
            nc.vector.memset(magic[:], MAGIC)
            nc.vector.memset(cdiag[:], 0.0)
            nc.vector.memset(zcol[:], 0.0)
            xpad_v = xpad[:].rearrange("p (h w) -> p h w", w=33)
            nc.vector.tensor_copy(xpad_v[:, :, 32], zcol[:])   # right pad col
            nc.vector.tensor_copy(xpad_v[:, 32, :], zcol[:])   # bottom pad row

            c_flat = c_all[:].rearrange("p b i -> p (b i)")
            cb_flat = c_bf[:].rearrange("p b i -> p (b i)")
            blog3 = blog[:].rearrange("p (b i) -> p b i", i=NUM)

            kv = km_d.rearrange("(c p) o -> p c o", p=128)
            with tc.tile_pool(name="kp", bufs=2) as kp:
                def kt_load(blk):
                    w = min(BLK, OO - blk * BLK)
                    t = kp.tile([128, 8, BLK], bf16, tag="kt")
                    nc.gpsimd.dma_start(t[:, :, 0:w],
                                        kv[:, :, blk * BLK:blk * BLK + w])
                    return t
                kt_q = [kt_load(0), kt_load(1)]

                # ---------- Phase 1: 2x2 SAME conv, per batch ----------
                with tc.tile_pool(name="psc", bufs=2, space="PSUM") as psc, \
                     tc.tile_pool(name="pst", bufs=2, space="PSUM") as pst:
                    for b in range(B_LOC):
                        for t in range(8):
                            pt = pst.tile([128, 128], bf16, tag="pt")
                            nc.tensor.transpose(pt[:], xin[:, b, t, :], eye[:])
                            src = pt[:].rearrange("p (a w) -> p a w", w=32)
                            dst = xpad_v[:, t * 4:(t + 1) * 4, 0:32]
                            if t % 2 == 0:
                                nc.vector.tensor_copy(dst, src)
                            else:
                                nc.scalar.copy(dst, src)
                        for hh in range(2):
                            pc = psc.tile([128, 512], f32, tag="pc")
                            for ti, (dh, dw) in enumerate(((0, 0), (0, 1), (1, 0), (1, 1))):
                                rhs = xpad_v[:, hh * 16 + dh: hh * 16 + dh + 16, dw:dw + 32]
                                nc.tensor.matmul(pc[:], wct[:, ti, :], rhs,
                                                 start=(ti == 0), stop=(ti == 3))
                            # raw-reshape gather: uT[t][pp, c] = conv[a, 8q+t, pp], c = 4a+q
                            pcv = pc[:].rearrange("p (a q t) -> p a q t", q=4, t=8)
                            for t in range(8):
                                src = pcv[:, :, :, t]
                                dst = uT[:, b, t, hh * 64:(hh + 1) * 64].rearrange(
                                    "p (a q) -> p a q", q=4)
                                if t % 2 == 0:
                                    nc.vector.tensor_copy(dst, src)
                                else:
                                    nc.scalar.copy(dst, src)

                # ---------- Phase 2+3: dense matmul (t0 fused) + routing in a
                # single pool region so routing can begin inside the dense tail
                with tc.tile_pool(name="scp", bufs=6) as scp, \
                     tc.tile_pool(name="rt", bufs=6) as rt, \
                     tc.tile_pool(name="psf", bufs=1, space="PSUM") as psf, \
                     tc.tile_pool(name="psm", bufs=2, space="PSUM") as psm, \
                     tc.tile_pool(name="po", bufs=2, space="PSUM") as po:

                    pfin_t = psf.tile([128, JK], f32)
                    creps = {}
                    scrs = {}

                    def emit_t0(b, i):
                        # o0(b,i) = 0.1-weighted colsum of u_hat block;
                        # drain o0 to bf16 on ACT, z per-channel dots on DVE
                        un = b * NUM + i
                        o0 = i * JK
                        pbc = po.tile([128, JK], f32, tag="pbc")
                        nc.tensor.matmul(pbc[:, 0:512], crep0[:],
                                         u_hat[:, b, o0:o0 + 512],
                                         start=True, stop=True)
                        nc.tensor.matmul(pbc[:, 512:JK], crep0[:],
                                         u_hat[:, b, o0 + 512:o0 + JK],
                                         start=True, stop=True)
                        scr = scp.tile([128, JK], bf16, tag="scr")
                        nc.scalar.copy(scr[:], pbc[:])
                        jnk = scp.tile([128, JK], bf16, tag="jnk")
                        nc.vector.scalar_tensor_tensor(
                            out=jnk[:],
                            in0=u_hat[:, b, o0:o0 + JK],
                            scalar=1.0, in1=scr[:],
                            op0=ALU.mult, op1=ALU.mult,
                            accum_out=z_all[:, un:un + 1])

                    def softmax_batched(bs, uniform_c, last):
                        nb = len(bs)
                        s0 = bs[0] * NUM
                        sl = slice(s0, s0 + nb * NUM)
                        bsl = slice(bs[0], bs[0] + nb)
                        if uniform_c:
                            nc.vector.tensor_scalar_mul(zc[:, sl], z_all[:, sl], 0.1)
                        else:
                            nc.vector.tensor_mul(zc[:, sl], z_all[:, sl], c_flat[:, sl])
                        ps = psm.tile([128, BLK], f32, tag="pm")
                        nc.tensor.matmul(ps[:, 0:nb * NUM], ones32[:], zc[:, sl],
                                         start=True, stop=True)
                        # b_logits = z * rsqrt(max(ss, eps)); rsqrt via quake
                        # seed + 2 Newton steps, all on DVE -> ACT only ever
                        # runs Exp/Copy (single activation table, no reloads)
                        nc.vector.tensor_scalar_max(ssm[:, sl], ps[:, 0:nb * NUM], EPS)
                        nc.vector.tensor_single_scalar(
                            tmp[:, sl].bitcast(i32), ssm[:, sl].bitcast(i32),
                            1, ALU.logical_shift_right)
                        nc.vector.tensor_sub(yr[:, sl].bitcast(i32), magic[:, sl],
                                             tmp[:, sl].bitcast(i32))
                        nc.vector.tensor_scalar_mul(ha[:, sl], ssm[:, sl], -0.5)
                        for _ in range(2):
                            nc.vector.tensor_mul(tmp[:, sl], yr[:, sl], yr[:, sl])
                            nc.vector.tensor_mul(tmp[:, sl], tmp[:, sl], ha[:, sl])
                            # yr = (tmp + 1.5) * yr, fused
                            nc.vector.scalar_tensor_tensor(
                                out=yr[:, sl], in0=tmp[:, sl], scalar=1.5,
                                in1=yr[:, sl], op0=ALU.add, op1=ALU.mult)
                        # b-logits are bounded by ||u_hat_c|| (Cauchy-Schwarz)
                        # ~ 25, so exp never overflows f32: skip max-subtract
                        nc.vector.tensor_mul(blog[:, sl], z_all[:, sl], yr[:, sl])
                        for b in bs:
                            nc.scalar.activation(eexp[:, b, :],
                                                 blog[:, b * NUM:(b + 1) * NUM],
                                                 AF.Exp,
                                                 accum_out=sume[:, b:b + 1])
                        nc.vector.reciprocal(rsum[:, bsl], sume[:, bsl])
                        for b in bs:
                            nc.vector.tensor_scalar_mul(
                                c_all[:, b, :], eexp[:, b, :], rsum[:, b:b + 1])
                        if last:
                            # scatter this group's c2 onto the cdiag diagonal
                            # (f32 -> bf16 cast in the copy)
                            cd_flat = cdiag[:].rearrange("p a b -> p (a b)")
                            nc.vector.tensor_copy(
                                cd_flat[:, s0 * (BN + 1):
                                        (s0 + nb * NUM - 1) * (BN + 1) + 1:BN + 1],
                                c_flat[:, sl])

                    # ---- software-pipelined t1: crep 2 ahead (DVE), MM pair +
                    # ACT drain 1 ahead, STT current.  Keeps the DVE STT stream
                    # dense across batch boundaries.
                    seq = [(b, i) for b in range(B_LOC) for i in range(NUM)]

                    def mm_stage(b, i):
                        o0 = i * JK
                        crep = rt.tile([128, 128], bf16, tag="crep")
                        nc.vector.tensor_scalar_mul(crep[:], ones_bf[:],
                                                    c_all[:, b, i:i + 1])
                        pbc = po.tile([128, JK], f32, tag="pbc")
                        nc.tensor.matmul(pbc[:, 0:512], crep[:],
                                         u_hat[:, b, o0:o0 + 512],
                                         start=True, stop=True)
                        nc.tensor.matmul(pbc[:, 512:JK], crep[:],
                                         u_hat[:, b, o0 + 512:o0 + JK],
                                         start=True, stop=True)
                        scr = scp.tile([128, JK], bf16, tag="scr")
                        nc.scalar.copy(scr[:], pbc[:])
                        scrs[(b, i)] = scr

                    def stt_stage(b, i):
                        un = b * NUM + i
                        o0 = i * JK
                        scr = scrs.pop((b, i))
                        jnk = scp.tile([128, JK], bf16, tag="jnk")
                        nc.vector.scalar_tensor_tensor(
                            out=jnk[:],
                            in0=u_hat[:, b, o0:o0 + JK],
                            scalar=1.0, in1=scr[:],
                            op0=ALU.mult, op1=ALU.mult,
                            accum_out=z_all[:, un:un + 1])

                    def prologue01():
                        # c1 for batches 0,1 + first t1 stages, emitted inside
                        # the dense tail so the t1 pipeline starts full
                        softmax_batched([0, 1], uniform_c=True, last=False)
                        mm_stage(*seq[0])

                    # ---------- dense sweep ----------
                    for blk in range(NBLK):
                        w = min(BLK, OO - blk * BLK)
                        kt = kt_q.pop(0)
                        for b in range(B_LOC):
                            pm = psm.tile([128, BLK], f32, tag="pm")
                            for ch in range(8):
                                nc.tensor.matmul(pm[:, 0:w], uT[:, b, ch, :],
                                                 kt[:, ch, 0:w],
                                                 start=(ch == 0), stop=(ch == 7))
                            dst = u_hat[:, b, blk * BLK:blk * BLK + w]
                            if (blk * B_LOC + b) % 2 == 0:
                                nc.vector.tensor_copy(dst, pm[:, 0:w])
                            else:
                                nc.scalar.copy(dst, pm[:, 0:w])
                            for i in TRIG.get(blk, []):
                                emit_t0(b, i)
                            if blk == NBLK - 1 and b == 1:
                                prologue01()
                        if blk + 2 < NBLK:
                            kt_q.append(kt_load(blk + 2))

                    # ---------- routing ----------
                    for k, (b, i) in enumerate(seq):
                        if k + 1 < len(seq):
                            mm_stage(*seq[k + 1])
                        stt_stage(b, i)
                        if (b, i) == (0, 2):
                            softmax_batched([2, 3], uniform_c=True, last=False)
                    softmax_batched([0, 1, 2, 3], uniform_c=False, last=True)

                    # final o with c2: lhsT = cdiag[:, r, :] -> capsule (b,i)
                    # accumulates onto PSUM row r of pfin_t; other rows get +0.
                    # Bunched after t1 so the PE runs hot back-to-back.
                    for b in range(B_LOC):
                        for i in range(NUM):
                            r = b * NUM + i
                            o0 = i * JK
                            nc.tensor.matmul(pfin_t[0:BN, 0:512],
                                             cdiag[:, r, :],
                                             u_hat[:, b, o0:o0 + 512],
                                             start=(r == 0), stop=(r == BN - 1),
                                             skip_group_check=True)
                            nc.tensor.matmul(pfin_t[0:BN, 512:JK],
                                             cdiag[:, r, :],
                                             u_hat[:, b, o0 + 512:o0 + JK],
                                             start=(r == 0), stop=(r == BN - 1),
                                             skip_group_check=True)
                    nc.scalar.copy(ofin[0:BN, :], pfin_t[0:BN, :])
                    nc.sync.dma_start(out_d.rearrange("b i jk -> (b i) jk"),
                                      ofin[0:BN, :])
    nc.compile()
    return nc


_NC_CACHE = None


def _get_nc():
    global _NC_CACHE
    if _NC_CACHE is None:
        _NC_CACHE = build_nc()
    return _NC_CACHE


def make_in_maps(u_vecs, W_conv, kernel):
    u_bf = np.asarray(u_vecs, dtype=ml_dtypes.bfloat16)
    wc_bf = np.ascontiguousarray(np.asarray(W_conv, dtype=ml_dtypes.bfloat16))
    km_bf = np.ascontiguousarray(np.asarray(kernel, dtype=ml_dtypes.bfloat16))
    eye = np.eye(C, dtype=ml_dtypes.bfloat16)
    return [
        {"u": np.ascontiguousarray(u_bf[ci * B_LOC:(ci + 1) * B_LOC]),
         "wc": wc_bf, "km": km_bf, "eye": eye}
        for ci in range(N_CORES)
    ]


def kernel(u_vecs, W_conv, kernel):
    nc = _get_nc()
    in_maps = make_in_maps(u_vecs, W_conv, kernel)
    res = run_bass_kernel_spmd(nc, in_maps, core_ids=list(range(N_CORES)))
    out = np.concatenate([r["out"] for r in res.results], axis=0)
    return out.reshape(B, NUM, D0, D1).astype(np.float32)


# revision 28
# speedup vs baseline: 1.0029x; 1.0029x over previous
import numpy as np
import ml_dtypes

import concourse.bacc as bacc
import concourse.mybir as mybir
import concourse.tile as tile
from concourse.bass_utils import run_bass_kernel_spmd

# Problem constants (hardcoded per harness contract)
B, H, W, C = 32, 32, 32, 128
NUM, D0, D1 = 10, 60, 16
JK = D0 * D1            # 960
OO = NUM * JK           # 9600
P = H * W               # 1024 contraction dim of the dense kernel
N_CORES = 8
B_LOC = B // N_CORES    # 4 batches per core
BLK = 512               # dense-kernel column block (1 PSUM bank of fp32)
NBLK = (OO + BLK - 1) // BLK  # 19, last block is 384 wide
BN = B_LOC * NUM        # 40
EPS = 1e-12
MAGIC = 0x5F3759DF      # quake rsqrt seed

# capsule i of a batch is fully drained once dense block TRIG[blk] lands
TRIG = {}
for i in range(NUM):
    _blk = ((i + 1) * JK + BLK - 1) // BLK - 1
    TRIG.setdefault(_blk, []).append(i)

f32 = mybir.dt.float32
i32 = mybir.dt.int32
f32r = mybir.dt.float32r
bf16 = mybir.dt.bfloat16
AF = mybir.ActivationFunctionType
ALU = mybir.AluOpType


def build_nc():
    nc = bacc.Bacc("TRN2", debug=False)
    u_d = nc.dram_tensor("u", (B_LOC, H, W, C), bf16, kind="ExternalInput").ap()
    wc_d = nc.dram_tensor("wc", (2, 2, C, C), bf16, kind="ExternalInput").ap()
    km_d = nc.dram_tensor("km", (P, OO), bf16, kind="ExternalInput").ap()
    eye_d = nc.dram_tensor("eye", (C, C), bf16, kind="ExternalInput").ap()
    out_d = nc.dram_tensor("out", (B_LOC, NUM, JK), f32, kind="ExternalOutput").ap()

    with tile.TileContext(nc) as tc:
        with tc.tile_pool(name="persist", bufs=1) as pers:
            u_hat = pers.tile([128, B_LOC, OO], bf16)       # [c, b, o]
            xin = pers.tile([128, B_LOC, 8, 128], bf16)     # raw u [sp, b, t, c]
            uT = pers.tile([128, B_LOC, 8, 128], bf16)      # lhsT chunks [p, b, chunk, c]
            wct = pers.tile([128, 4, C], bf16)              # conv taps [ci, tap, co]
            eye = pers.tile([128, C], bf16)
            ones_bf = pers.tile([128, 128], bf16)
            crep0 = pers.tile([128, 128], bf16)             # uniform c = 0.1
            ones32 = pers.tile([128, 128], f32r)
            c_all = pers.tile([128, B_LOC, NUM], f32)
            c_bf = pers.tile([128, B_LOC, NUM], bf16)
            z_all = pers.tile([128, BN], f32)
            zc = pers.tile([128, BN], f32r)
            ssm = pers.tile([128, BN], f32)
            magic = pers.tile([128, BN], i32)
            yr = pers.tile([128, BN], f32)
            ha = pers.tile([128, BN], f32)
            tmp = pers.tile([128, BN], f32)
            blog = pers.tile([128, BN], f32)
            eexp = pers.tile([128, B_LOC, NUM], f32)
            nmax = pers.tile([128, B_LOC], f32)
            sume = pers.tile([128, B_LOC], f32)
            rsum = pers.tile([128, B_LOC], f32)
            ofin = pers.tile([128, JK], f32)                # rows (b*NUM+i) hold final o
            # cdiag[:, r, :] = c2 column (b,i)=r at position r, zeros elsewhere;
            # used as t2 lhsT so capsule r lands on PSUM row r (base partition 0)
            cdiag = pers.tile([128, BN, BN], bf16)

            xpad = pers.tile([128, 33 * 33], bf16)
            zcol = pers.tile([128, 33], bf16)

            # input DMAs first, spread across queues so descriptor generation
            # overlaps; batch 0 first so conv can start as early as possible
            nc.sync.dma_start(eye[:], eye_d)
            uv = u_d.rearrange("b h w c -> b (h w) c").rearrange(
                "b (t sp) c -> b sp t c", sp=128)
            nc.gpsimd.dma_start(xin[:, 0], uv[0])
            nc.scalar.dma_start(wct[:], wc_d.rearrange("dh dw ci co -> ci (dh dw) co"))
            for _b in range(1, B_LOC):
                nc.gpsimd.dma_start(xin[:, _b], uv[_b])

            nc.vector.memset(ones_bf[:], 1.0)
            nc.vector.tensor_copy(ones32[:], ones_bf[:])
            nc.vector.memset(crep0[:], 0.1)
            nc.vector.memset(magic[:], MAGIC)
            nc.vector.memset(cdiag[:], 0.0)
            nc.vector.memset(zcol[:], 0.0)
            xpad_v = xpad[:].rearrange("p (h w) -> p h w", w=33)
            nc.vector.tensor_copy(xpad_v[:, :, 32], zcol[:])   # right pad col
            nc.vector.tensor_copy(xpad_v[:, 32, :], zcol[:])   # bottom pad row

            c_flat = c_all[:].rearrange("p b i -> p (b i)")
            cb_flat = c_bf[:].rearrange("p b i -> p (b i)")
            blog3 = blog[:].rearrange("p (b i) -> p b i", i=NUM)

            kv = km_d.rearrange("(c p) o -> p c o", p=128)
            with tc.tile_pool(name="kp", bufs=2) as kp:
                def kt_load(blk):
                    w = min(BLK, OO - blk * BLK)
                    t = kp.tile([128, 8, BLK], bf16, tag="kt")
                    nc.gpsimd.dma_start(t[:, :, 0:w],
                                        kv[:, :, blk * BLK:blk * BLK + w])
                    return t
                kt_q = [kt_load(0), kt_load(1)]

                # ---------- Phase 1: 2x2 SAME conv, per batch ----------
                with tc.tile_pool(name="psc", bufs=2, space="PSUM") as psc, \
                     tc.tile_pool(name="pst", bufs=2, space="PSUM") as pst:
                    for b in range(B_LOC):
                        for t in range(8):
                            pt = pst.tile([128, 128], bf16, tag="pt")
                            nc.tensor.transpose(pt[:], xin[:, b, t, :], eye[:])
                            src = pt[:].rearrange("p (a w) -> p a w", w=32)
                            dst = xpad_v[:, t * 4:(t + 1) * 4, 0:32]
                            if t % 2 == 0:
                                nc.vector.tensor_copy(dst, src)
                            else:
                                nc.scalar.copy(dst, src)
                        for hh in range(2):
                            pc = psc.tile([128, 512], f32, tag="pc")
                            for ti, (dh, dw) in enumerate(((0, 0), (0, 1), (1, 0), (1, 1))):
                                rhs = xpad_v[:, hh * 16 + dh: hh * 16 + dh + 16, dw:dw + 32]
                                nc.tensor.matmul(pc[:], wct[:, ti, :], rhs,
                                                 start=(ti == 0), stop=(ti == 3))
                            # raw-reshape gather: uT[t][pp, c] = conv[a, 8q+t, pp], c = 4a+q
                            pcv = pc[:].rearrange("p (a q t) -> p a q t", q=4, t=8)
                            for t in range(8):
                                src = pcv[:, :, :, t]
                                dst = uT[:, b, t, hh * 64:(hh + 1) * 64].rearrange(
                                    "p (a q) -> p a q", q=4)
                                if t % 2 == 0:
                                    nc.vector.tensor_copy(dst, src)
                                else:
                                    nc.scalar.copy(dst, src)

                # ---------- Phase 2+3: dense matmul (t0 fused) + routing in a
                # single pool region so routing can begin inside the dense tail
                with tc.tile_pool(name="scp", bufs=6) as scp, \
                     tc.tile_pool(name="rt", bufs=6) as rt, \
                     tc.tile_pool(name="psf", bufs=1, space="PSUM") as psf, \
                     tc.tile_pool(name="psm", bufs=2, space="PSUM") as psm, \
                     tc.tile_pool(name="po", bufs=2, space="PSUM") as po:

                    pfin_t = psf.tile([128, JK], f32)
                    creps = {}
                    scrs = {}

                    def emit_t0(b, i):
                        # o0(b,i) = 0.1-weighted colsum of u_hat block;
                        # drain o0 to bf16 on ACT, z per-channel dots on DVE
                        un = b * NUM + i
                        o0 = i * JK
                        pbc = po.tile([128, JK], f32, tag="pbc")
                        nc.tensor.matmul(pbc[:, 0:512], crep0[:],
                                         u_hat[:, b, o0:o0 + 512],
                                         start=True, stop=True)
                        nc.tensor.matmul(pbc[:, 512:JK], crep0[:],
                                         u_hat[:, b, o0 + 512:o0 + JK],
                                         start=True, stop=True)
                        scr = scp.tile([128, JK], bf16, tag="scr")
                        nc.scalar.copy(scr[:], pbc[:])
                        jnk = scp.tile([128, JK], bf16, tag="jnk")
                        nc.vector.scalar_tensor_tensor(
                            out=jnk[:],
                            in0=u_hat[:, b, o0:o0 + JK],
                            scalar=1.0, in1=scr[:],
                            op0=ALU.mult, op1=ALU.mult,
                            accum_out=z_all[:, un:un + 1])

                    def softmax_batched(bs, uniform_c, last):
                        nb = len(bs)
                        s0 = bs[0] * NUM
                        sl = slice(s0, s0 + nb * NUM)
                        bsl = slice(bs[0], bs[0] + nb)
                        if uniform_c:
                            nc.vector.tensor_scalar_mul(zc[:, sl], z_all[:, sl], 0.1)
                        else:
                            nc.vector.tensor_mul(zc[:, sl], z_all[:, sl], c_flat[:, sl])
                        ps = psm.tile([128, BLK], f32, tag="pm")
                        nc.tensor.matmul(ps[:, 0:nb * NUM], ones32[:], zc[:, sl],
                                         start=True, stop=True)
                        # b_logits = z * rsqrt(max(ss, eps)); rsqrt via quake
                        # seed + 2 Newton steps, all on DVE -> ACT only ever
                        # runs Exp/Copy (single activation table, no reloads)
                        nc.vector.tensor_scalar_max(ssm[:, sl], ps[:, 0:nb * NUM], EPS)
                        nc.vector.tensor_single_scalar(
                            tmp[:, sl].bitcast(i32), ssm[:, sl].bitcast(i32),
                            1, ALU.logical_shift_right)
                        nc.vector.tensor_sub(yr[:, sl].bitcast(i32), magic[:, sl],
                                             tmp[:, sl].bitcast(i32))
                        nc.vector.tensor_scalar_mul(ha[:, sl], ssm[:, sl], -0.5)
                        for _ in range(2):
                            nc.vector.tensor_mul(tmp[:, sl], yr[:, sl], yr[:, sl])
                            nc.vector.tensor_mul(tmp[:, sl], tmp[:, sl], ha[:, sl])
                            # yr = (tmp + 1.5) * yr, fused
                            nc.vector.scalar_tensor_tensor(
                                out=yr[:, sl], in0=tmp[:, sl], scalar=1.5,
                                in1=yr[:, sl], op0=ALU.add, op1=ALU.mult)
                        # b-logits are bounded by ||u_hat_c|| (Cauchy-Schwarz)
                        # ~ 25, so exp never overflows f32: skip max-subtract
                        nc.vector.tensor_mul(blog[:, sl], z_all[:, sl], yr[:, sl])
                        for b in bs:
                            nc.scalar.activation(eexp[:, b, :],
                                                 blog[:, b * NUM:(b + 1) * NUM],
                                                 AF.Exp,
                                                 accum_out=sume[:, b:b + 1])
                        nc.vector.reciprocal(rsum[:, bsl], sume[:, bsl])
                        for b in bs:
                            nc.vector.tensor_scalar_mul(
                                c_all[:, b, :], eexp[:, b, :], rsum[:, b:b + 1])
                        if last:
                            # scatter this group's c2 onto the cdiag diagonal
                            # (f32 -> bf16 cast in the copy)
                            cd_flat = cdiag[:].rearrange("p a b -> p (a b)")
                            nc.vector.tensor_copy(
                                cd_flat[:, s0 * (BN + 1):
                                        (s0 + nb * NUM - 1) * (BN + 1) + 1:BN + 1],
                                c_flat[:, sl])

                    # ---- software-pipelined t1: crep 2 ahead (DVE), MM pair +
                    # ACT drain 1 ahead, STT current.  Keeps the DVE STT stream
                    # dense across batch boundaries.
                    seq = [(b, i) for b in range(B_LOC) for i in range(NUM)]

                    def mm_stage(b, i):
                        o0 = i * JK
                        crep = rt.tile([128, 128], bf16, tag="crep")
                        nc.vector.tensor_scalar_mul(crep[:], ones_bf[:],
                                                    c_all[:, b, i:i + 1])
                        pbc = po.tile([128, JK], f32, tag="pbc")
                        nc.tensor.matmul(pbc[:, 0:512], crep[:],
                                         u_hat[:, b, o0:o0 + 512],
                                         start=True, stop=True)
                        nc.tensor.matmul(pbc[:, 512:JK], crep[:],
                                         u_hat[:, b, o0 + 512:o0 + JK],
                                         start=True, stop=True)
                        scr = scp.tile([128, JK], bf16, tag="scr")
                        nc.scalar.copy(scr[:], pbc[:])
                        scrs[(b, i)] = scr

                    def stt_stage(b, i):
                        un = b * NUM + i
                        o0 = i * JK
                        scr = scrs.pop((b, i))
                        jnk = scp.tile([128, JK], bf16, tag="jnk")
                        nc.vector.scalar_tensor_tensor(
                            out=jnk[:],
                            in0=u_hat[:, b, o0:o0 + JK],
                            scalar=1.0, in1=scr[:],
                            op0=ALU.mult, op1=ALU.mult,
                            accum_out=z_all[:, un:un + 1])

                    def prologue01():
                        # c1 for batches 0,1 + first t1 stages, emitted inside
                        # the dense tail so the t1 pipeline starts full
                        softmax_batched([0, 1], uniform_c=True, last=False)
                        mm_stage(*seq[0])

                    # ---------- dense sweep ----------
                    for blk in range(NBLK):
                        w = min(BLK, OO - blk * BLK)
                        kt = kt_q.pop(0)
                        for b in range(B_LOC):
                            pm = psm.tile([128, BLK], f32, tag="pm")
                            for ch in range(8):
                                nc.tensor.matmul(pm[:, 0:w], uT[:, b, ch, :],
                                                 kt[:, ch, 0:w],
                                                 start=(ch == 0), stop=(ch == 7))
                            dst = u_hat[:, b, blk * BLK:blk * BLK + w]
                            if (blk * B_LOC + b) % 2 == 0:
                                nc.vector.tensor_copy(dst, pm[:, 0:w])
                            else:
                                nc.scalar.copy(dst, pm[:, 0:w])
                            for i in TRIG.get(blk, []):
                                emit_t0(b, i)
                            if blk == NBLK - 1 and b == 1:
                                prologue01()
                            if blk == NBLK - 1 and b == 3:
                                softmax_batched([2, 3], uniform_c=True,
                                                last=False)
                        if blk + 2 < NBLK:
                            kt_q.append(kt_load(blk + 2))

                    # ---------- routing ----------
                    for k, (b, i) in enumerate(seq):
                        if k + 1 < len(seq):
                            mm_stage(*seq[k + 1])
                        stt_stage(b, i)
                    softmax_batched([0, 1, 2, 3], uniform_c=False, last=True)

                    # final o with c2: lhsT = cdiag[:, r, :] -> capsule (b,i)
                    # accumulates onto PSUM row r of pfin_t; other rows get +0.
                    # Bunched after t1 so the PE runs hot back-to-back.
                    for b in range(B_LOC):
                        for i in range(NUM):
                            r = b * NUM + i
                            o0 = i * JK
                            nc.tensor.matmul(pfin_t[0:BN, 0:512],
                                             cdiag[:, r, :],
                                             u_hat[:, b, o0:o0 + 512],
                                             start=(r == 0), stop=(r == BN - 1),
                                             skip_group_check=True)
                            nc.tensor.matmul(pfin_t[0:BN, 512:JK],
                                             cdiag[:, r, :],
                                             u_hat[:, b, o0 + 512:o0 + JK],
                                             start=(r == 0), stop=(r == BN - 1),
                                             skip_group_check=True)
                    nc.scalar.copy(ofin[0:BN, :], pfin_t[0:BN, :])
                    nc.sync.dma_start(out_d.rearrange("b i jk -> (b i) jk"),
                                      ofin[0:BN, :])
    nc.compile()
    return nc


_NC_CACHE = None


def _get_nc():
    global _NC_CACHE
    if _NC_CACHE is None:
        _NC_CACHE = build_nc()
    return _NC_CACHE


def make_in_maps(u_vecs, W_conv, kernel):
    u_bf = np.asarray(u_vecs, dtype=ml_dtypes.bfloat16)
    wc_bf = np.ascontiguousarray(np.asarray(W_conv, dtype=ml_dtypes.bfloat16))
    km_bf = np.ascontiguousarray(np.asarray(kernel, dtype=ml_dtypes.bfloat16))
    eye = np.eye(C, dtype=ml_dtypes.bfloat16)
    return [
        {"u": np.ascontiguousarray(u_bf[ci * B_LOC:(ci + 1) * B_LOC]),
         "wc": wc_bf, "km": km_bf, "eye": eye}
        for ci in range(N_CORES)
    ]


def kernel(u_vecs, W_conv, kernel):
    nc = _get_nc()
    in_maps = make_in_maps(u_vecs, W_conv, kernel)
    res = run_bass_kernel_spmd(nc, in_maps, core_ids=list(range(N_CORES)))
    out = np.concatenate([r["out"] for r in res.results], axis=0)
    return out.reshape(B, NUM, D0, D1).astype(np.float32)


# revision 29
# speedup vs baseline: 1.0211x; 1.0181x over previous
import numpy as np
import ml_dtypes

import concourse.bacc as bacc
import concourse.mybir as mybir
import concourse.tile as tile
from concourse.bass_utils import run_bass_kernel_spmd

# Problem constants (hardcoded per harness contract)
B, H, W, C = 32, 32, 32, 128
NUM, D0, D1 = 10, 60, 16
JK = D0 * D1            # 960
OO = NUM * JK           # 9600
P = H * W               # 1024 contraction dim of the dense kernel
N_CORES = 8
B_LOC = B // N_CORES    # 4 batches per core
BLK = 512               # dense-kernel column block (1 PSUM bank of fp32)
NBLK = (OO + BLK - 1) // BLK  # 19, last block is 384 wide
BN = B_LOC * NUM        # 40
EPS = 1e-12
MAGIC = 0x5F3759DF      # quake rsqrt seed

# capsule i of a batch is fully drained once dense block TRIG[blk] lands
TRIG = {}
for i in range(NUM):
    _blk = ((i + 1) * JK + BLK - 1) // BLK - 1
    TRIG.setdefault(_blk, []).append(i)

f32 = mybir.dt.float32
i32 = mybir.dt.int32
f32r = mybir.dt.float32r
bf16 = mybir.dt.bfloat16
AF = mybir.ActivationFunctionType
ALU = mybir.AluOpType


def build_nc():
    nc = bacc.Bacc("TRN2", debug=False)
    u_d = nc.dram_tensor("u", (B_LOC, H, W, C), bf16, kind="ExternalInput").ap()
    wc_d = nc.dram_tensor("wc", (2, 2, C, C), bf16, kind="ExternalInput").ap()
    km_d = nc.dram_tensor("km", (P, OO), bf16, kind="ExternalInput").ap()
    eye_d = nc.dram_tensor("eye", (C, C), bf16, kind="ExternalInput").ap()
    out_d = nc.dram_tensor("out", (B_LOC, NUM, JK), f32, kind="ExternalOutput").ap()

    with tile.TileContext(nc) as tc:
        with tc.tile_pool(name="persist", bufs=1) as pers:
            u_hat = pers.tile([128, B_LOC, OO], bf16)       # [c, b, o]
            xin = pers.tile([128, B_LOC, 8, 128], bf16)     # raw u [sp, b, t, c]
            uT = pers.tile([128, B_LOC, 8, 128], bf16)      # lhsT chunks [p, b, chunk, c]
            wct = pers.tile([128, 4, C], bf16)              # conv taps [ci, tap, co]
            eye = pers.tile([128, C], bf16)
            ones_bf = pers.tile([128, 128], bf16)
            crep0 = pers.tile([128, 128], bf16)             # uniform c = 0.1
            ones32 = pers.tile([128, 128], f32r)
            c_all = pers.tile([128, B_LOC, NUM], f32)
            c_bf = pers.tile([128, B_LOC, NUM], bf16)
            z_all = pers.tile([128, BN], f32)
            zc = pers.tile([128, BN], f32r)
            ssm = pers.tile([128, BN], f32)
            magic = pers.tile([128, BN], i32)
            yr = pers.tile([128, BN], f32)
            ha = pers.tile([128, BN], f32)
            tmp = pers.tile([128, BN], f32)
            blog = pers.tile([128, BN], f32)
            eexp = pers.tile([128, B_LOC, NUM], f32)
            nmax = pers.tile([128, B_LOC], f32)
            sume = pers.tile([128, B_LOC], f32)
            rsum = pers.tile([128, B_LOC], f32)
            ofin = pers.tile([128, 2, JK], f32)             # two 20-row groups of final o
            # cdiag[:, r, :] = c2 column (b,i)=r at position r, zeros elsewhere;
            # used as t2 lhsT so capsule r lands on PSUM row r (base partition 0)
            cdiag = pers.tile([128, BN, BN], bf16)

            xpad = pers.tile([128, 33 * 33], bf16)
            zcol = pers.tile([128, 33], bf16)

            # input DMAs first, spread across queues so descriptor generation
            # overlaps; batch 0 first so conv can start as early as possible
            nc.sync.dma_start(eye[:], eye_d)
            uv = u_d.rearrange("b h w c -> b (h w) c").rearrange(
                "b (t sp) c -> b sp t c", sp=128)
            nc.gpsimd.dma_start(xin[:, 0], uv[0])
            nc.scalar.dma_start(wct[:], wc_d.rearrange("dh dw ci co -> ci (dh dw) co"))
            for _b in range(1, B_LOC):
                nc.gpsimd.dma_start(xin[:, _b], uv[_b])

            nc.vector.memset(ones_bf[:], 1.0)
            nc.vector.tensor_copy(ones32[:], ones_bf[:])
            nc.vector.memset(crep0[:], 0.1)
            nc.vector.memset(magic[:], MAGIC)
            nc.vector.memset(cdiag[:], 0.0)
            nc.vector.memset(zcol[:], 0.0)
            xpad_v = xpad[:].rearrange("p (h w) -> p h w", w=33)
            nc.vector.tensor_copy(xpad_v[:, :, 32], zcol[:])   # right pad col
            nc.vector.tensor_copy(xpad_v[:, 32, :], zcol[:])   # bottom pad row

            c_flat = c_all[:].rearrange("p b i -> p (b i)")
            cb_flat = c_bf[:].rearrange("p b i -> p (b i)")
            blog3 = blog[:].rearrange("p (b i) -> p b i", i=NUM)

            kv = km_d.rearrange("(c p) o -> p c o", p=128)
            with tc.tile_pool(name="kp", bufs=2) as kp:
                def kt_load(blk):
                    w = min(BLK, OO - blk * BLK)
                    t = kp.tile([128, 8, BLK], bf16, tag="kt")
                    nc.gpsimd.dma_start(t[:, :, 0:w],
                                        kv[:, :, blk * BLK:blk * BLK + w])
                    return t
                kt_q = [kt_load(0), kt_load(1)]

                # ---------- Phase 1: 2x2 SAME conv, per batch ----------
                with tc.tile_pool(name="psc", bufs=2, space="PSUM") as psc, \
                     tc.tile_pool(name="pst", bufs=2, space="PSUM") as pst:
                    for b in range(B_LOC):
                        for t in range(8):
                            pt = pst.tile([128, 128], bf16, tag="pt")
                            nc.tensor.transpose(pt[:], xin[:, b, t, :], eye[:])
                            src = pt[:].rearrange("p (a w) -> p a w", w=32)
                            dst = xpad_v[:, t * 4:(t + 1) * 4, 0:32]
                            if t % 2 == 0:
                                nc.vector.tensor_copy(dst, src)
                            else:
                                nc.scalar.copy(dst, src)
                        for hh in range(2):
                            pc = psc.tile([128, 512], f32, tag="pc")
                            for ti, (dh, dw) in enumerate(((0, 0), (0, 1), (1, 0), (1, 1))):
                                rhs = xpad_v[:, hh * 16 + dh: hh * 16 + dh + 16, dw:dw + 32]
                                nc.tensor.matmul(pc[:], wct[:, ti, :], rhs,
                                                 start=(ti == 0), stop=(ti == 3))
                            # raw-reshape gather: uT[t][pp, c] = conv[a, 8q+t, pp], c = 4a+q
                            pcv = pc[:].rearrange("p (a q t) -> p a q t", q=4, t=8)
                            for t in range(8):
                                src = pcv[:, :, :, t]
                                dst = uT[:, b, t, hh * 64:(hh + 1) * 64].rearrange(
                                    "p (a q) -> p a q", q=4)
                                if t % 2 == 0:
                                    nc.vector.tensor_copy(dst, src)
                                else:
                                    nc.scalar.copy(dst, src)

                # ---------- Phase 2+3: dense matmul (t0 fused) + routing in a
                # single pool region so routing can begin inside the dense tail
                with tc.tile_pool(name="scp", bufs=6) as scp, \
                     tc.tile_pool(name="rt", bufs=6) as rt, \
                     tc.tile_pool(name="psf", bufs=1, space="PSUM") as psf, \
                     tc.tile_pool(name="psm", bufs=2, space="PSUM") as psm, \
                     tc.tile_pool(name="po", bufs=2, space="PSUM") as po:

                    pfin_t = psf.tile([128, JK], f32)
                    creps = {}
                    scrs = {}

                    def emit_t0(b, i):
                        # o0(b,i) = 0.1-weighted colsum of u_hat block;
                        # drain o0 to bf16 on ACT, z per-channel dots on DVE
                        un = b * NUM + i
                        o0 = i * JK
                        pbc = po.tile([128, JK], f32, tag="pbc")
                        nc.tensor.matmul(pbc[:, 0:512], crep0[:],
                                         u_hat[:, b, o0:o0 + 512],
                                         start=True, stop=True)
                        nc.tensor.matmul(pbc[:, 512:JK], crep0[:],
                                         u_hat[:, b, o0 + 512:o0 + JK],
                                         start=True, stop=True)
                        scr = scp.tile([128, JK], bf16, tag="scr")
                        nc.scalar.copy(scr[:], pbc[:])
                        jnk = scp.tile([128, JK], bf16, tag="jnk")
                        nc.vector.scalar_tensor_tensor(
                            out=jnk[:],
                            in0=u_hat[:, b, o0:o0 + JK],
                            scalar=1.0, in1=scr[:],
                            op0=ALU.mult, op1=ALU.mult,
                            accum_out=z_all[:, un:un + 1])

                    def softmax_batched(bs, uniform_c, last):
                        nb = len(bs)
                        s0 = bs[0] * NUM
                        sl = slice(s0, s0 + nb * NUM)
                        bsl = slice(bs[0], bs[0] + nb)
                        if uniform_c:
                            nc.vector.tensor_scalar_mul(zc[:, sl], z_all[:, sl], 0.1)
                        else:
                            nc.vector.tensor_mul(zc[:, sl], z_all[:, sl], c_flat[:, sl])
                        ps = psm.tile([128, BLK], f32, tag="pm")
                        nc.tensor.matmul(ps[:, 0:nb * NUM], ones32[:], zc[:, sl],
                                         start=True, stop=True)
                        # b_logits = z * rsqrt(max(ss, eps)); rsqrt via quake
                        # seed + 2 Newton steps, all on DVE -> ACT only ever
                        # runs Exp/Copy (single activation table, no reloads)
                        nc.vector.tensor_scalar_max(ssm[:, sl], ps[:, 0:nb * NUM], EPS)
                        nc.vector.tensor_single_scalar(
                            tmp[:, sl].bitcast(i32), ssm[:, sl].bitcast(i32),
                            1, ALU.logical_shift_right)
                        nc.vector.tensor_sub(yr[:, sl].bitcast(i32), magic[:, sl],
                                             tmp[:, sl].bitcast(i32))
                        nc.vector.tensor_scalar_mul(ha[:, sl], ssm[:, sl], -0.5)
                        for _ in range(2):
                            nc.vector.tensor_mul(tmp[:, sl], yr[:, sl], yr[:, sl])
                            nc.vector.tensor_mul(tmp[:, sl], tmp[:, sl], ha[:, sl])
                            # yr = (tmp + 1.5) * yr, fused
                            nc.vector.scalar_tensor_tensor(
                                out=yr[:, sl], in0=tmp[:, sl], scalar=1.5,
                                in1=yr[:, sl], op0=ALU.add, op1=ALU.mult)
                        # b-logits are bounded by ||u_hat_c|| (Cauchy-Schwarz)
                        # ~ 25, so exp never overflows f32: skip max-subtract
                        nc.vector.tensor_mul(blog[:, sl], z_all[:, sl], yr[:, sl])
                        for b in bs:
                            nc.scalar.activation(eexp[:, b, :],
                                                 blog[:, b * NUM:(b + 1) * NUM],
                                                 AF.Exp,
                                                 accum_out=sume[:, b:b + 1])
                        nc.vector.reciprocal(rsum[:, bsl], sume[:, bsl])
                        for b in bs:
                            nc.vector.tensor_scalar_mul(
                                c_all[:, b, :], eexp[:, b, :], rsum[:, b:b + 1])
                        if last:
                            # scatter this group's c2 onto the cdiag diagonal
                            # (f32 -> bf16 cast in the copy)
                            cd_flat = cdiag[:].rearrange("p a b -> p (a b)")
                            nc.vector.tensor_copy(
                                cd_flat[:, s0 * (BN + 1):
                                        (s0 + nb * NUM - 1) * (BN + 1) + 1:BN + 1],
                                c_flat[:, sl])

                    # ---- software-pipelined t1: crep 2 ahead (DVE), MM pair +
                    # ACT drain 1 ahead, STT current.  Keeps the DVE STT stream
                    # dense across batch boundaries.
                    seq = [(b, i) for b in range(B_LOC) for i in range(NUM)]

                    def mm_stage(b, i):
                        o0 = i * JK
                        crep = rt.tile([128, 128], bf16, tag="crep")
                        nc.vector.tensor_scalar_mul(crep[:], ones_bf[:],
                                                    c_all[:, b, i:i + 1])
                        pbc = po.tile([128, JK], f32, tag="pbc")
                        nc.tensor.matmul(pbc[:, 0:512], crep[:],
                                         u_hat[:, b, o0:o0 + 512],
                                         start=True, stop=True)
                        nc.tensor.matmul(pbc[:, 512:JK], crep[:],
                                         u_hat[:, b, o0 + 512:o0 + JK],
                                         start=True, stop=True)
                        scr = scp.tile([128, JK], bf16, tag="scr")
                        nc.scalar.copy(scr[:], pbc[:])
                        scrs[(b, i)] = scr

                    def stt_stage(b, i):
                        un = b * NUM + i
                        o0 = i * JK
                        scr = scrs.pop((b, i))
                        jnk = scp.tile([128, JK], bf16, tag="jnk")
                        nc.vector.scalar_tensor_tensor(
                            out=jnk[:],
                            in0=u_hat[:, b, o0:o0 + JK],
                            scalar=1.0, in1=scr[:],
                            op0=ALU.mult, op1=ALU.mult,
                            accum_out=z_all[:, un:un + 1])

                    def prologue01():
                        # c1 for batches 0,1 + first t1 stages, emitted inside
                        # the dense tail so the t1 pipeline starts full
                        softmax_batched([0, 1], uniform_c=True, last=False)
                        mm_stage(*seq[0])

                    # ---------- dense sweep ----------
                    for blk in range(NBLK):
                        w = min(BLK, OO - blk * BLK)
                        kt = kt_q.pop(0)
                        for b in range(B_LOC):
                            pm = psm.tile([128, BLK], f32, tag="pm")
                            for ch in range(8):
                                nc.tensor.matmul(pm[:, 0:w], uT[:, b, ch, :],
                                                 kt[:, ch, 0:w],
                                                 start=(ch == 0), stop=(ch == 7))
                            dst = u_hat[:, b, blk * BLK:blk * BLK + w]
                            if (blk * B_LOC + b) % 2 == 0:
                                nc.vector.tensor_copy(dst, pm[:, 0:w])
                            else:
                                nc.scalar.copy(dst, pm[:, 0:w])
                            for i in TRIG.get(blk, []):
                                emit_t0(b, i)
                            if blk == NBLK - 1 and b == 1:
                                prologue01()
                            if blk == NBLK - 1 and b == 3:
                                softmax_batched([2, 3], uniform_c=True,
                                                last=False)
                        if blk + 2 < NBLK:
                            kt_q.append(kt_load(blk + 2))

                    # ---------- routing ----------
                    for k, (b, i) in enumerate(seq):
                        if k + 1 < len(seq):
                            mm_stage(*seq[k + 1])
                        stt_stage(b, i)
                    softmax_batched([0, 1, 2, 3], uniform_c=False, last=True)

                    # final o with c2 in two 20-row groups: lhsT =
                    # cdiag[:, r, g*20:(g+1)*20] puts capsule (b,i) on PSUM row
                    # r-20g of its group tile; other rows get +0.  Group 0
                    # drains and DMAs out while group 1's matmuls still run.
                    out_rows = out_d.rearrange("b i jk -> (b i) jk")
                    HB = 2 * NUM
                    pfin_b = po.tile([128, JK], f32, tag="pbc")
                    for g, pf in ((0, pfin_t), (1, pfin_b)):
                        for b in (2 * g, 2 * g + 1):
                            for i in range(NUM):
                                r = b * NUM + i
                                o0 = i * JK
                                lhs = cdiag[:, r, g * HB:(g + 1) * HB]
                                st = (r == g * HB)
                                sp = (r == g * HB + HB - 1)
                                nc.tensor.matmul(pf[0:HB, 0:512], lhs,
                                                 u_hat[:, b, o0:o0 + 512],
                                                 start=st, stop=sp,
                                                 skip_group_check=True)
                                nc.tensor.matmul(pf[0:HB, 512:JK], lhs,
                                                 u_hat[:, b, o0 + 512:o0 + JK],
                                                 start=st, stop=sp,
                                                 skip_group_check=True)
                        nc.scalar.copy(ofin[0:HB, g, :], pf[0:HB, :])
                        nc.sync.dma_start(out_rows[g * HB:(g + 1) * HB],
                                          ofin[0:HB, g, :])
    nc.compile()
    return nc


_NC_CACHE = None


def _get_nc():
    global _NC_CACHE
    if _NC_CACHE is None:
        _NC_CACHE = build_nc()
    return _NC_CACHE


def make_in_maps(u_vecs, W_conv, kernel):
    u_bf = np.asarray(u_vecs, dtype=ml_dtypes.bfloat16)
    wc_bf = np.ascontiguousarray(np.asarray(W_conv, dtype=ml_dtypes.bfloat16))
    km_bf = np.ascontiguousarray(np.asarray(kernel, dtype=ml_dtypes.bfloat16))
    eye = np.eye(C, dtype=ml_dtypes.bfloat16)
    return [
        {"u": np.ascontiguousarray(u_bf[ci * B_LOC:(ci + 1) * B_LOC]),
         "wc": wc_bf, "km": km_bf, "eye": eye}
        for ci in range(N_CORES)
    ]


def kernel(u_vecs, W_conv, kernel):
    nc = _get_nc()
    in_maps = make_in_maps(u_vecs, W_conv, kernel)
    res = run_bass_kernel_spmd(nc, in_maps, core_ids=list(range(N_CORES)))
    out = np.concatenate([r["out"] for r in res.results], axis=0)
    return out.reshape(B, NUM, D0, D1).astype(np.float32)
